# revision 19
# baseline (speedup 1.0000x reference)
"""Distributed Trainium2 kernel for the GNN message-passing model.

Self-contained: host-side structural prep (sharding, edge sort, index
remap) + Bass/Tile SPMD kernel across 8 NeuronCores.

Math (see reference):
  logits = MLP(x1); m = 0.15 + 0.55*onehot(argmax(logits))
  r1 = (m@W1s.sum(-1))*x2 + m@bp1
  g1 = relu(Dh A Dh (r1@gcn1_w) + gcn1_b); g1 = (m@W12)*g1 + 2e-4*(r1@W13)
  r2 = (m@W2.sum(-1))*g1 + m@bp2
  g2 = relu(Dh A Dh (r2@gcn2_w) + gcn2_b)
  out = log_softmax(g2@fc_w + fc_b)
where Dh = diag(deg^-1/2), deg = in-degree over dst.

Distribution: nodes sharded contiguously over 8 cores. Per GCN layer the
scaled features h' = Dh*h are AllGathered in fp8 (in node chunks, so
comm overlaps the producer pipeline); each core gathers h'[src] for
edges whose dst it owns via indirect DMA and scatter-reduces them with
one-hot matmuls on the TensorEngine (PSUM accumulation per dst block).
The one-hot scatter matrices are generated on-chip (iota==drel on DVE)
instead of streamed from HBM; fp8 DoubleRow matmuls contract 256 edges
per instruction for GCN layer 1.
"""

import numpy as np

P = 128
TAU_HI = 0.7
TAU_LO = 0.15  # (1-0.7)/2


class _Cfg:
    def __init__(self, N, E, F1=768, H=512, G1=256, G2=32, FOUT=40, C=7):
        self.NC = 8
        self.N = N
        self.E = E
        self.NLOC_RAW = N // self.NC
        self.NB = -(-self.NLOC_RAW // P)          # node blocks per core
        self.NLOC = self.NB * P
        assert self.NB % C == 0, (self.NB, C)
        self.C = C                                 # allgather chunks
        self.BPC = self.NB // C                    # blocks per chunk
        self.CH = self.BPC * P                     # chunk nodes
        self.TR = self.NC * self.NLOC              # gathered table rows
        self.CHR = self.NC * self.CH               # rows per chunk in table
        self.F1, self.H, self.G1, self.G2, self.FOUT = F1, H, G1, G2, FOUT
        self.KF1 = F1 // P                         # 6 k-tiles
        self.KH = H // P                           # 4
        self.KG1 = G1 // P                         # 2
        self.SPL1 = max(1, C - 1)                  # L1 round-A src chunks
        self.SPL2 = max(1, C - 3)                  # L2 round-A src chunks
        self.NFREE = min(448, self.CH)             # front free-dim unit
        assert self.CH % self.NFREE == 0
        self.FU = self.CH // self.NFREE            # free units per chunk


CFG_FULL = dict(N=50000, E=800000)


def _to_bf16(x):
    import ml_dtypes
    return np.asarray(x, np.float32).astype(ml_dtypes.bfloat16)


def _to_f8(x):
    import ml_dtypes
    return np.clip(np.asarray(x, np.float32), -240.0, 240.0).astype(
        ml_dtypes.float8_e4m3fn)


def _row_of_node(v, cfg):
    """Gathered-table row for global node id v (vectorized)."""
    c = v // cfg.NLOC_RAW
    s = v - c * cfg.NLOC_RAW
    k = s // cfg.CH
    return k * cfg.CHR + c * cfg.CH + (s - k * cfg.CH)


def host_prep(inputs, cfg):
    """Returns (in_maps, sched). sched is baked into the built graph and
    must be identical for every core (SPMD)."""
    x1 = np.asarray(inputs["x1"], np.float32)
    x2 = np.asarray(inputs["x2"], np.float32)
    ei = np.asarray(inputs["edge_index"])
    src = ei[0].astype(np.int64)
    dst = ei[1].astype(np.int64)
    N, E, NC = cfg.N, cfg.E, cfg.NC
    assert x1.shape[0] == N and src.shape[0] == E

    deg = np.bincount(dst, minlength=N).astype(np.float64)
    dinv = np.where(deg > 0, deg ** -0.5, 0.0).astype(np.float32)
    sdeg = np.sqrt(deg).astype(np.float32)  # 1/dinv where deg>0 else 0

    # ---- per-core edge partition by dst owner, sorted by dst block ----
    owner = dst // cfg.NLOC_RAW
    dloc = dst - owner * cfg.NLOC_RAW
    dblk = dloc // P
    drel_all = (dloc - dblk * P).astype(np.float32)
    rows_all = _row_of_node(src, cfg).astype(np.int32)

    per_core = []
    cnt = np.zeros((NC, cfg.NB), np.int64)
    for c in range(NC):
        sel = np.where(owner == c)[0]
        order = np.argsort(dblk[sel], kind="stable")
        sel = sel[order]
        b_of = dblk[sel]
        bounds = np.searchsorted(b_of, np.arange(cfg.NB + 1))
        lists = []
        for b in range(cfg.NB):
            idxs = sel[bounds[b]:bounds[b + 1]]
            lists.append((rows_all[idxs], drel_all[idxs]))
            cnt[c, b] = len(idxs)
        per_core.append(lists)

    # Two uniform cross-core layouts, each split in 2 rounds by src chunk
    # (round boundary = which AllGather chunks the gathers depend on).
    # pair=True keeps per-block bundle counts even so DoubleRow matmuls
    # can consume bundle pairs belonging to the same dst block.
    def build_layout(split_chunk, pad_mult, pair):
        bounds_k = [0, split_chunk * cfg.CHR, cfg.C * cfg.CHR]
        layout = dict(rounds=[])
        for r in range(2):
            lo, hi = bounds_k[r], bounds_k[r + 1]
            cntr = np.zeros((NC, cfg.NB), np.int64)
            per_rc = []
            for c in range(NC):
                pc = []
                for b in range(cfg.NB):
                    rows, rel = per_core[c][b]
                    m = (rows >= lo) & (rows < hi)
                    pc.append((rows[m] - lo, rel[m]))
                    cntr[c, b] = int(m.sum())
                per_rc.append(pc)
            # layouts must be identical on every core: use the max count
            mx = cntr.max(axis=0)
            if pair:
                Kb = 2 * np.maximum(1, -(-mx // (2 * P))).astype(np.int64)
            else:
                Kb = np.maximum(1, -(-mx // P)).astype(np.int64)
            nb_round = int(Kb.sum())
            pad = (-nb_round) % pad_mult
            nb_round += pad
            b_of = np.concatenate([np.repeat(np.arange(cfg.NB), Kb),
                                   np.full(pad, cfg.NB - 1)])
            first = np.zeros(nb_round, bool)
            last = np.zeros(nb_round, bool)
            off = 0
            for b in range(cfg.NB):
                first[off] = True
                e = off + int(Kb[b])
                if b == cfg.NB - 1:
                    e = nb_round
                last[e - 1] = True
                off += int(Kb[b])
            layout["rounds"].append(dict(Kb=Kb, nblocks=nb_round, b_of=b_of,
                                         first=first, last=last,
                                         per_rc=per_rc))
        return layout

    lay1 = build_layout(cfg.SPL1, 16, True)
    lay2 = build_layout(cfg.SPL2, 32, False)

    # pack per-core gather indices + fp8 one-hot scatter matrices
    def pack_layout(layout, c):
        idxs, Ss = [], []
        eye = np.eye(P, dtype=np.float32)
        zrow = np.zeros((1, P), np.float32)
        eyez = np.concatenate([eye, zrow], 0)  # row P = padding (all-zero)
        for rr in layout["rounds"]:
            sbs = rr["nblocks"] // 8
            idx = np.zeros((sbs * P, 8), np.int32)
            drl = np.full((sbs * P, 8), P, np.int32)
            g = 0
            for b in range(cfg.NB):
                rows, rel = rr["per_rc"][c][b]
                n = len(rows)
                nblk = int(rr["Kb"][b])
                if b == cfg.NB - 1:
                    nblk = rr["nblocks"] - g
                for j in range(nblk):
                    s, jj = g // 8, g % 8
                    e0 = j * P
                    m = min(P, max(0, n - e0))
                    if m > 0:
                        idx[s * P:s * P + m, jj] = rows[e0:e0 + m]
                        drl[s * P:s * P + m, jj] = rel[e0:e0 + m].astype(
                            np.int32)
                    g += 1
            idxs.append(idx)
            S = eyez[drl.reshape(sbs * P, 8)]  # [sbs*P, 8, P]
            Ss.append(_to_f8(S.reshape(sbs * P, 8 * P)))
        return (np.concatenate(idxs, axis=0), np.concatenate(Ss, axis=0))

    sched = dict(lay1=[dict(nblocks=r["nblocks"], b_of=r["b_of"],
                            first=r["first"], last=r["last"])
                       for r in lay1["rounds"]],
                 lay2=[dict(nblocks=r["nblocks"], b_of=r["b_of"],
                            first=r["first"], last=r["last"])
                       for r in lay2["rounds"]])

    # ---- weights ----
    w1 = np.asarray(inputs["mlp_w1"], np.float32)
    w2 = np.asarray(inputs["mlp_w2"], np.float32)
    w3 = np.asarray(inputs["mlp_w3"], np.float32)
    b1 = np.asarray(inputs["mlp_b1"], np.float32)
    b2 = np.asarray(inputs["mlp_b2"], np.float32)
    b3 = np.asarray(inputs["mlp_b3"], np.float32)
    W1s = np.asarray(inputs["W1"], np.float32).sum(-1)
    W12 = np.asarray(inputs["W12"], np.float32)
    W13 = np.asarray(inputs["W13"], np.float32) * 2e-4
    bp1 = np.asarray(inputs["bp1"], np.float32)
    W2s = np.asarray(inputs["W2"], np.float32).sum(-1)
    bp2 = np.asarray(inputs["bp2"], np.float32)
    g1w = np.asarray(inputs["gcn1_w"], np.float32)
    g1b = np.asarray(inputs["gcn1_b"], np.float32)
    g2w = np.asarray(inputs["gcn2_w"], np.float32)
    g2b = np.asarray(inputs["gcn2_b"], np.float32)
    fcw = np.asarray(inputs["fc_w"], np.float32)
    fcb = np.asarray(inputs["fc_b"], np.float32)

    sched["bp1_nz"] = bool(np.any(bp1 != 0))
    sched["bp2_nz"] = bool(np.any(bp2 != 0))
    sched["g1b_nz"] = bool(np.any(g1b != 0))
    sched["g2b_nz"] = bool(np.any(g2b != 0))
    sched["fcb_nz"] = bool(np.any(fcb != 0))
    sched["b3_nz"] = bool(np.any(b3 != 0))

    def pack_lhsT_dr(w, KT, MT):
        # DoubleRow stationary layout: [(kp, m, h)] blocks; k-tile pair
        # (2kp, 2kp+1) of output block m sits adjacent.
        o = np.zeros((P, KT * MT * P), np.float32)
        for kp in range(KT // 2):
            for m in range(MT):
                for h in range(2):
                    o[:, ((kp * MT + m) * 2 + h) * P:
                         ((kp * MT + m) * 2 + h + 1) * P] = \
                        w[(2 * kp + h) * P:(2 * kp + h + 1) * P,
                          m * P:(m + 1) * P]
        return _to_f8(o)

    def pack_rhs(w, KT, F, conv=_to_bf16):
        o = np.zeros((P, KT * F), np.float32)
        for k in range(KT):
            o[:, k * F:(k + 1) * F] = w[k * P:(k + 1) * P, :]
        return conv(o)

    def pack_k3(w, F):
        o = np.zeros((4, F), np.float32)
        o[:3] = w
        return _to_bf16(o)

    w1_p = pack_lhsT_dr(w1, cfg.KF1, cfg.KH)
    w2_p = pack_lhsT_dr(w2, cfg.KH, cfg.KH)
    # logits rhs padded to 16 cols per k-tile so DoubleRow pair stride
    # is 16B-aligned
    w3_p = pack_rhs(np.pad(w3, ((0, 0), (0, 13))), cfg.KH, 16, _to_f8)
    b1_p = b1.reshape(cfg.KH, P).T.copy()
    b2_p = b2.reshape(cfg.KH, P).T.copy()
    b3_p = np.pad(b3, (0, 1)).reshape(1, 4).repeat(P, 0).copy()
    # fused rhs for h1' and z: [768, 256+256]
    gz = np.concatenate([g1w, W13], axis=1)
    gz_p = pack_rhs(gz, cfg.KF1, 2 * cfg.G1, _to_f8)
    g2w_p = pack_rhs(g2w, cfg.KG1, cfg.G2)
    fcw_p = _to_bf16(fcw)
    W1s_p = pack_k3(W1s, cfg.F1)
    bp1_p = pack_k3(bp1, cfg.F1)
    W12_p = pack_k3(W12, cfg.G1)
    W2s_p = pack_k3(W2s, cfg.G1)
    bp2_p = pack_k3(bp2, cfg.G1)
    g1b_p = _to_bf16(g1b.reshape(1, cfg.G1))
    g2b_p = _to_bf16(g2b.reshape(1, cfg.G2))
    fcb_p = _to_bf16(fcb.reshape(1, cfg.FOUT))

    in_maps = []
    for c in range(NC):
        lo = c * cfg.NLOC_RAW
        hi = lo + cfg.NLOC_RAW
        x1T = np.zeros((cfg.F1, cfg.NLOC), np.float32)
        x1T[:, :cfg.NLOC_RAW] = x1[lo:hi].T
        x2T = np.zeros((cfg.F1, cfg.NLOC), np.float32)
        x2T[:, :cfg.NLOC_RAW] = x2[lo:hi].T
        dinv_t = np.zeros((P, cfg.NB), np.float32)
        dinv_t.T.reshape(-1)[:cfg.NLOC_RAW] = dinv[lo:hi]
        sdeg_r = np.zeros((1, cfg.NLOC), np.float32)
        sdeg_r[0, :cfg.NLOC_RAW] = sdeg[lo:hi]

        ident_np = _to_bf16(np.eye(P, dtype=np.float32))
        idx1, Sm1 = pack_layout(lay1, c)
        idx2, Sm2 = pack_layout(lay2, c)
        im = {
            "ident": ident_np,
            "x1T": _to_f8(x1T), "x2T": _to_f8(x2T),
            "idx1": idx1, "Sm1": Sm1, "idx2": idx2, "Sm2": Sm2,
            "dinv_t": dinv_t, "sdeg_r": _to_bf16(sdeg_r),
            "w1": w1_p, "w2": w2_p, "w3": w3_p,
            "b1": b1_p, "b2": b2_p, "b3": b3_p,
            "gz": gz_p, "g2w": g2w_p, "fcw": fcw_p,
            "W1s": W1s_p, "bp1": bp1_p, "W12": W12_p, "W2s": W2s_p,
            "bp2": bp2_p, "g1b": g1b_p, "g2b": g2b_p, "fcb": fcb_p,
        }
        in_maps.append(im)
    return in_maps, sched


def build(cfg, sched, debug=False):
    import concourse.bacc as bacc
    import concourse.bass as bass
    import concourse.mybir as mybir
    import concourse.tile as tile

    dt = mybir.dt
    AF = mybir.ActivationFunctionType
    OP = mybir.AluOpType
    AX = mybir.AxisListType
    DR = mybir.MatmulPerfMode.DoubleRow

    nc = bacc.Bacc("TRN2", target_bir_lowering=False, debug=debug)

    NB, C, BPC, CH, NLOC, TR, CHR = (cfg.NB, cfg.C, cfg.BPC, cfg.CH,
                                     cfg.NLOC, cfg.TR, cfg.CHR)
    F1, H, G1, G2, FOUT = cfg.F1, cfg.H, cfg.G1, cfg.G2, cfg.FOUT
    KF1, KH, KG1 = cfg.KF1, cfg.KH, cfg.KG1
    NF, FU = cfg.NFREE, cfg.FU
    L1A, L1B = sched["lay1"]
    L2A, L2B = sched["lay2"]
    SB1 = (L1A["nblocks"] + L1B["nblocks"]) // 8
    SB2T = (L2A["nblocks"] + L2B["nblocks"]) // 8
    SPL1, SPL2 = cfg.SPL1, cfg.SPL2

    bf = dt.bfloat16
    f8 = dt.float8e4
    f32 = dt.float32

    import os
    L1W = 1 if os.environ.get("K_NO_DR_SCAT") else 2
    # L2 table stays bf16: 32-byte fp8 indirect-gather rows corrupt on HW
    f8t2 = dt.bfloat16

    dd = {}

    def din(name, shape, dtype):
        dd[name] = nc.declare_dram_parameter(name, list(shape), dtype,
                                             isOutput=False)
        return dd[name]

    x1T_d = din("x1T", [F1, NLOC], f8)
    x2T_d = din("x2T", [F1, NLOC], f8)
    idx1_d = din("idx1", [SB1 * P, 8], dt.int32)
    Sm1_d = din("Sm1", [SB1 * P, 8 * P], f8)
    idx2_d = din("idx2", [SB2T * P, 8], dt.int32)
    Sm2_d = din("Sm2", [SB2T * P, 8 * P], f8)
    dinv_d = din("dinv_t", [P, NB], f32)
    sdeg_d = din("sdeg_r", [1, NLOC], bf)
    w1_d = din("w1", [P, KF1 * KH * P], f8)
    w2_d = din("w2", [P, KH * KH * P], f8)
    w3_d = din("w3", [P, KH * 16], f8)
    b1_d = din("b1", [P, KH], f32)
    b2_d = din("b2", [P, KH], f32)
    b3_d = din("b3", [P, 4], f32)
    gz_d = din("gz", [P, KF1 * 2 * G1], f8)
    g2w_d = din("g2w", [P, KG1 * G2], bf)
    fcw_d = din("fcw", [G2, FOUT], bf)
    W1s_d = din("W1s", [4, F1], bf)
    bp1_d = din("bp1", [4, F1], bf)
    W12_d = din("W12", [4, G1], bf)
    W2s_d = din("W2s", [4, G1], bf)
    bp2_d = din("bp2", [4, G1], bf)
    g1b_d = din("g1b", [1, G1], bf)
    g2b_d = din("g2b", [1, G2], bf)
    fcb_d = din("fcb", [1, FOUT], bf)
    ident_d = din("ident", [P, P], bf)
    out_d = nc.declare_dram_parameter("out", [NLOC, FOUT], f32, isOutput=True)
    DBG = bool(os.environ.get("K_DEBUG_DUMP"))
    if DBG:
        dbg_r2_d = nc.declare_dram_parameter("dbg_r2", [NLOC, G1], f32,
                                             isOutput=True)
        dbg_g2_d = nc.declare_dram_parameter("dbg_g2", [NLOC, G2], f32,
                                             isOutput=True)

    with tile.TileContext(nc) as tc:
        with (
            tc.tile_pool(name="const", bufs=1) as cp,
            tc.tile_pool(name="front", bufs=2) as fp,
            tc.tile_pool(name="scat", bufs=3) as sp,
            tc.tile_pool(name="fin", bufs=2) as qp,
            tc.tile_pool(name="psG", bufs=2, space="PSUM") as psG,
            tc.tile_pool(name="psS", bufs=2, space="PSUM") as psS,
            tc.tile_pool(name="psB", bufs=2, space="PSUM") as psB,
            tc.tile_pool(name="psT", bufs=2, space="PSUM") as psT,
            tc.tile_pool(name="dram", bufs=1, space="DRAM") as dp,
        ):
            def load(dr, shape, dtype, name):
                t = cp.tile(shape, dtype, tag=name)
                nc.sync.dma_start(out=t[:, :], in_=dr[:, :])
                return t

            w1_s = load(w1_d, [P, KF1 * KH * P], f8, "w1")
            w2_s = load(w2_d, [P, KH * KH * P], f8, "w2")
            w3_s = load(w3_d, [P, KH * 16], f8, "w3")
            b1_s = load(b1_d, [P, KH], f32, "b1")
            b2_s = load(b2_d, [P, KH], f32, "b2")
            b3_s = load(b3_d, [P, 4], f32, "b3")
            gz_s = load(gz_d, [P, KF1 * 2 * G1], f8, "gz")
            g2w_s = load(g2w_d, [P, KG1 * G2], bf, "g2w")
            fcw_s = load(fcw_d, [G2, FOUT], bf, "fcw")
            W1s_s = load(W1s_d, [4, F1], bf, "W1s")
            bp1_s = load(bp1_d, [4, F1], bf, "bp1")
            W12_s = load(W12_d, [4, G1], bf, "W12")
            W2s_s = load(W2s_d, [4, G1], bf, "W2s")
            bp2_s = load(bp2_d, [4, G1], bf, "bp2")
            g1b_s = load(g1b_d, [1, G1], bf, "g1b")
            g2b_s = load(g2b_d, [1, G2], bf, "g2b")
            fcb_s = load(fcb_d, [1, FOUT], bf, "fcb")
            dinv_s = load(dinv_d, [P, NB], f32, "dinv")
            sdeg_s = load(sdeg_d, [1, NLOC], bf, "sdeg")

            ident = load(ident_d, [P, P], bf, "ident")
            ones1 = cp.tile([1, P], bf, tag="ones1")
            nc.vector.memset(ones1[:, :], 1.0)

            mT_s = cp.tile([4, NLOC], bf, tag="mT")
            out_acc = cp.tile([P, NB * FOUT], f32, tag="oacc")
            z_s = cp.tile([P, NB * G1], bf, tag="z")

            h1b_all = dp.tile([NLOC, G1], f8, tag="h1b")
            ag1_after = {2: (0, 3), 5: (3, 6), C - 1: (6, C)}
            h2bA = dp.tile([SPL2 * CH, G2], f8t2, tag="h2bA")
            h2bB = dp.tile([(C - SPL2) * CH, G2], f8t2, tag="h2bB")
            h1gA = dp.tile([SPL1 * CHR, G1], f8, tag="h1gA")
            h1gB = dp.tile([(C - SPL1) * CHR, G1], f8, tag="h1gB")
            h2gA = dp.tile([SPL2 * CHR, G2], f8t2, tag="h2gA")
            h2gB = dp.tile([(C - SPL2) * CHR, G2], f8t2, tag="h2gB")
            aggA_d = dp.tile([NLOC, G1], bf, tag="aggA")
            agg2_s = cp.tile([P, NB * G2], bf, tag="agg2")

            def pair_k(t, width, kp, c0, cn):
                """[P, 2, cn] view of k-tile pair (2kp, 2kp+1), cols
                c0:c0+cn, from a [P, K*width] k-major tile."""
                return t[:, :].rearrange("p (k n) -> p k n", n=width)[
                    :, 2 * kp:2 * kp + 2, c0:c0 + cn]

            # ================= FRONT (per chunk) =================
            for k in range(C):
                n0 = k * CH
                x1c = fp.tile([P, KF1 * CH], f8, tag="x1c")
                nc.sync.dma_start(
                    out=x1c[:, :].rearrange("p (a n) -> p a n", n=CH),
                    in_=x1T_d[:, n0:n0 + CH].rearrange("(a p) n -> p a n", p=P))
                x2c = fp.tile([P, KF1 * CH], f8, tag="x2c", bufs=1)
                nc.sync.dma_start(
                    out=x2c[:, :].rearrange("p (a n) -> p a n", n=CH),
                    in_=x2T_d[:, n0:n0 + CH].rearrange("(a p) n -> p a n", p=P))

                h1T = fp.tile([P, KH * CH], f8, tag="h1T", bufs=1)
                for u in range(FU):
                    for m in range(KH):
                        ps = psG.tile([P, 512], f32, tag="g")
                        for kp in range(KF1 // 2):
                            nc.tensor.matmul(
                                ps[:, :NF],
                                lhsT=w1_s[:, :].rearrange(
                                    "p (a x) -> p a x", x=P)[
                                    :, (kp * KH + m) * 2:(kp * KH + m) * 2 + 2, :],
                                rhs=pair_k(x1c, CH, kp, u * NF, NF),
                                start=(kp == 0), stop=(kp == KF1 // 2 - 1),
                                perf_mode=DR)
                        nc.scalar.activation(
                            h1T[:, m * CH + u * NF:m * CH + u * NF + NF],
                            ps[:, :NF], AF.Relu, bias=b1_s[:, m:m + 1])
                h2T = fp.tile([P, KH * CH], f8, tag="h2T", bufs=1)
                for u in range(FU):
                    for m in range(KH):
                        ps = psG.tile([P, 512], f32, tag="g")
                        for kp in range(KH // 2):
                            nc.tensor.matmul(
                                ps[:, :NF],
                                lhsT=w2_s[:, :].rearrange(
                                    "p (a x) -> p a x", x=P)[
                                    :, (kp * KH + m) * 2:(kp * KH + m) * 2 + 2, :],
                                rhs=pair_k(h1T, CH, kp, u * NF, NF),
                                start=(kp == 0), stop=(kp == KH // 2 - 1),
                                perf_mode=DR)
                        nc.scalar.activation(
                            h2T[:, m * CH + u * NF:m * CH + u * NF + NF],
                            ps[:, :NF], AF.Relu, bias=b2_s[:, m:m + 1])

                mmc = fp.tile([P, BPC * 3], bf, tag="mmc")
                for nb in range(BPC):
                    psl = psB.tile([P, 512], f32, tag="b")
                    for kp in range(KH // 2):
                        nc.tensor.matmul(
                            psl[:, :16],
                            lhsT=pair_k(h2T, CH, kp, nb * P, P),
                            rhs=pair_k(w3_s, 16, kp, 0, 16),
                            start=(kp == 0), stop=(kp == KH // 2 - 1),
                            perf_mode=DR)
                    lg = fp.tile([P, 3], f32, tag="lg")
                    if sched["b3_nz"]:
                        nc.vector.tensor_add(lg[:, :], psl[:, :3], b3_s[:, :3])
                    else:
                        nc.vector.tensor_copy(lg[:, :], psl[:, :3])
                    rmax = fp.tile([P, 1], f32, tag="rmax")
                    nc.vector.reduce_max(rmax[:, :], lg[:, :], axis=AX.X)
                    mm = fp.tile([P, 3], bf, tag="mm")
                    nc.vector.tensor_scalar(
                        mm[:, :], lg[:, :], rmax[:, :1], None, OP.is_equal)
                    nc.scalar.activation(mmc[:, nb * 3:(nb + 1) * 3],
                                         mm[:, :], AF.Copy,
                                         bias=TAU_LO, scale=TAU_HI - TAU_LO)
                for nb in range(BPC):
                    b_glob = k * BPC + nb
                    pst = psT.tile([P, P], bf, tag="t")
                    nc.tensor.transpose(pst[:3, :],
                                        mmc[:, nb * 3:(nb + 1) * 3],
                                        ident[:, :])
                    nc.vector.tensor_copy(
                        mT_s[:3, b_glob * P:(b_glob + 1) * P], pst[:3, :])

                r1T = fp.tile([P, KF1 * CH], f8, tag="r1T")
                for u in range(FU):
                    for f in range(KF1):
                        psr = psG.tile([P, 512], f32, tag="g")
                        nc.tensor.matmul(
                            psr[:, :NF], lhsT=W1s_s[:3, f * P:(f + 1) * P],
                            rhs=mT_s[:3, n0 + u * NF:n0 + u * NF + NF],
                            start=True, stop=True)
                        if sched["bp1_nz"]:
                            psr2 = psB.tile([P, 512], f32, tag="b")
                            nc.tensor.matmul(
                                psr2[:, :NF], lhsT=bp1_s[:3, f * P:(f + 1) * P],
                                rhs=mT_s[:3, n0 + u * NF:n0 + u * NF + NF],
                                start=True, stop=True)
                            tmp = fp.tile([P, NF], f32, tag="r1tmp")
                            nc.vector.tensor_mul(
                                tmp[:, :], psr[:, :NF],
                                x2c[:, f * CH + u * NF:f * CH + u * NF + NF])
                            nc.vector.tensor_add(
                                r1T[:, f * CH + u * NF:f * CH + u * NF + NF],
                                tmp[:, :], psr2[:, :NF])
                        else:
                            nc.vector.tensor_mul(
                                r1T[:, f * CH + u * NF:f * CH + u * NF + NF],
                                psr[:, :NF],
                                x2c[:, f * CH + u * NF:f * CH + u * NF + NF])

                # fused h1' | z: [r1 @ (g1w | 2e-4*W13)] per node block
                for nb in range(BPC):
                    b_glob = k * BPC + nb
                    psh = psG.tile([P, 512], f32, tag="g")
                    for kp in range(KF1 // 2):
                        nc.tensor.matmul(
                            psh[:, :],
                            lhsT=pair_k(r1T, CH, kp, nb * P, P),
                            rhs=pair_k(gz_s, 2 * G1, kp, 0, 2 * G1),
                            start=(kp == 0), stop=(kp == KF1 // 2 - 1),
                            perf_mode=DR)
                    h1p = fp.tile([P, G1], f8, tag="h1p")
                    nc.scalar.activation(h1p[:, :], psh[:, :G1], AF.Copy,
                                         scale=dinv_s[:, b_glob:b_glob + 1])
                    nc.scalar.dma_start(
                        out=h1b_all[(n0 + nb * P):(n0 + (nb + 1) * P), :],
                        in_=h1p[:, :])
                    nc.scalar.activation(
                        z_s[:, b_glob * G1:(b_glob + 1) * G1],
                        psh[:, G1:2 * G1], AF.Copy)

                if k in ag1_after:
                    k0, k1 = ag1_after[k]
                    agt = (h1gA[k0 * CHR:k1 * CHR, :] if k1 <= SPL1 else
                           h1gB[0:CHR, :])
                    nc.gpsimd.collective_compute(
                        "AllGather", OP.bypass,
                        replica_groups=[list(range(cfg.NC))],
                        ins=[h1b_all[k0 * CH:k1 * CH, :].opt()],
                        outs=[agt.opt()])

            # ================= LAYER 1 scatter (2 rounds) =================
            ps_by_b = {}

            def l1_finalize(b):
                psb = ps_by_b.pop(b)
                if sched["g1b_nz"]:
                    nc.tensor.matmul(
                        psb[:, :], lhsT=sdeg_s[:1, b * P:(b + 1) * P],
                        rhs=g1b_s[:1, :], start=False, stop=True,
                        skip_group_check=True)
                g1r = qp.tile([P, G1], bf, tag="g1r", bufs=3)
                nc.scalar.activation(g1r[:, :], psb[:, :], AF.Relu,
                                     scale=dinv_s[:, b:b + 1])
                psmw = psB.tile([P, 512], f32, tag="b")
                nc.tensor.matmul(psmw[:, :G1],
                                 lhsT=mT_s[:3, b * P:(b + 1) * P],
                                 rhs=W12_s[:3, :], start=True, stop=True)
                nc.tensor.matmul(psmw[:, G1:2 * G1],
                                 lhsT=mT_s[:3, b * P:(b + 1) * P],
                                 rhs=W2s_s[:3, :], start=True, stop=True)
                # stage psum->SBUF on Scalar, then alternate the
                # elementwise chain between Vector and GpSimd per block
                wmw = qp.tile([P, 2 * G1], bf, tag="wmw", bufs=3)
                nc.scalar.activation(wmw[:, :], psmw[:, :2 * G1], AF.Copy)
                eng = nc.vector if (b % 2 == 0) else nc.gpsimd
                g1t = qp.tile([P, G1], bf, tag="g1t", bufs=3)
                eng.tensor_mul(g1t[:, :], g1r[:, :], wmw[:, :G1])
                g1v = qp.tile([P, G1], bf, tag="g1v", bufs=3)
                eng.tensor_add(g1v[:, :], g1t[:, :],
                               z_s[:, b * G1:(b + 1) * G1])
                r2 = qp.tile([P, G1], bf, tag="r2", bufs=3)
                if sched["bp2_nz"]:
                    psm3 = psB.tile([P, 512], f32, tag="b")
                    nc.tensor.matmul(psm3[:, :G1],
                                     lhsT=mT_s[:3, b * P:(b + 1) * P],
                                     rhs=bp2_s[:3, :], start=True, stop=True)
                    r2u = qp.tile([P, G1], bf, tag="r2u")
                    nc.vector.tensor_mul(r2u[:, :], g1v[:, :],
                                         psmw[:, G1:2 * G1])
                    r2v = qp.tile([P, G1], bf, tag="r2v")
                    nc.vector.tensor_add(r2v[:, :], r2u[:, :], psm3[:, :G1])
                    nc.vector.tensor_scalar(r2[:, :], r2v[:, :],
                                            dinv_s[:, b:b + 1], None, OP.mult)
                else:
                    nc.vector.scalar_tensor_tensor(
                        out=r2[:, :], in0=g1v[:, :],
                        scalar=dinv_s[:, b:b + 1],
                        in1=wmw[:, G1:2 * G1], op0=OP.mult, op1=OP.mult)
                if DBG:
                    r2d = qp.tile([P, G1], f32, tag="r2d", bufs=2)
                    nc.vector.tensor_copy(r2d[:, :], r2[:, :])
                    nc.scalar.dma_start(
                        out=dbg_r2_d[b * P:(b + 1) * P, :], in_=r2d[:, :])
                r2T = qp.tile([P, KG1 * P], bf, tag="r2T", bufs=3)
                for f in range(KG1):
                    pst = psT.tile([P, P], bf, tag="t")
                    nc.tensor.transpose(pst[:, :], r2[:, f * P:(f + 1) * P],
                                        ident[:, :])
                    if f == 0:
                        nc.vector.tensor_copy(r2T[:, f * P:(f + 1) * P],
                                              pst[:, :])
                    else:
                        nc.scalar.activation(r2T[:, f * P:(f + 1) * P],
                                             pst[:, :], AF.Copy)
                psh2 = psB.tile([P, 512], f32, tag="b")
                for f in range(KG1):
                    nc.tensor.matmul(
                        psh2[:, :G2], lhsT=r2T[:, f * P:(f + 1) * P],
                        rhs=g2w_s[:, f * G2:(f + 1) * G2],
                        start=(f == 0), stop=(f == KG1 - 1))
                h2p = qp.tile([P, G2], f8t2, tag="h2p", bufs=3)
                if b % 2 == 0:
                    nc.vector.tensor_copy(h2p[:, :], psh2[:, :G2])
                else:
                    nc.scalar.activation(h2p[:, :], psh2[:, :G2], AF.Copy)
                if b < SPL2 * BPC:
                    nc.scalar.dma_start(
                        out=h2bA[b * P:(b + 1) * P, :], in_=h2p[:, :])
                    if b == SPL2 * BPC - 1:
                        nc.gpsimd.collective_compute(
                            "AllGather", OP.bypass,
                            replica_groups=[list(range(cfg.NC))],
                            ins=[h2bA[:, :].opt()], outs=[h2gA[:, :].opt()])
                else:
                    bb = b - SPL2 * BPC
                    nc.scalar.dma_start(
                        out=h2bB[bb * P:(bb + 1) * P, :], in_=h2p[:, :])
                    if b == NB - 1:
                        nc.gpsimd.collective_compute(
                            "AllGather", OP.bypass,
                            replica_groups=[list(range(cfg.NC))],
                            ins=[h2bB[:, :].opt()], outs=[h2gB[:, :].opt()])

            def l1_round(meta, sb_base, table, is_b):
                for s_loc in range(meta["nblocks"] // 16):
                    r0 = (sb_base + s_loc * 2) * P
                    gt = sp.tile([P, 16 * G1], f8, tag="gt1")
                    ix = sp.tile([P, 16], dt.int32, tag="ix1")
                    nc.sync.dma_start(
                        out=ix[:, :].rearrange("p (a e) -> p a e", e=8),
                        in_=idx1_d[r0:r0 + 2 * P, :]
                            .rearrange("(a p) e -> p a e", p=P))
                    nc.gpsimd.indirect_dma_start(
                        out=gt[:, :], out_offset=None, in_=table[:, :],
                        in_offset=bass.IndirectOffsetOnAxis(ap=ix[:, :],
                                                            axis=0))
                    Ssb = sp.tile([P, 16 * P], f8, tag="S1", bufs=2)
                    nc.scalar.dma_start(
                        out=Ssb[:, :].rearrange("p (a v) -> p a v", v=8 * P),
                        in_=Sm1_d[r0:r0 + 2 * P, :]
                            .rearrange("(a p) v -> p a v", p=P))
                    for j2 in range(8):
                        g = s_loc * 16 + j2 * 2
                        b = int(meta["b_of"][g])
                        first = bool(meta["first"][g])
                        last = bool(meta["last"][g + 1])
                        if first:
                            psb = psS.tile([P, G1], f32, tag="agg",
                                           name="agg1")
                            ps_by_b[b] = psb
                            if is_b:
                                rel = sp.tile([P, G1], bf, tag="rel")
                                nc.sync.dma_start(
                                    out=rel[:, :],
                                    in_=aggA_d[b * P:(b + 1) * P, :])
                                nc.tensor.matmul(psb[:, :], lhsT=ident[:, :],
                                                 rhs=rel[:, :], start=True,
                                                 stop=False)
                        psb = ps_by_b[b]
                        stop = last and (not sched["g1b_nz"] if is_b else True)
                        nc.tensor.matmul(
                            psb[:, :],
                            lhsT=Ssb[:, :].rearrange(
                                "p (a v) -> p a v", v=P)[
                                :, 2 * j2:2 * j2 + 2, :],
                            rhs=gt[:, :].rearrange(
                                "p (a v) -> p a v", v=G1)[
                                :, 2 * j2:2 * j2 + 2, :],
                            start=(first and not is_b), stop=stop,
                            perf_mode=DR)
                        if not last:
                            continue
                        if not is_b:
                            tmpa = qp.tile([P, G1], bf, tag="tmpa")
                            nc.scalar.activation(tmpa[:, :],
                                                 ps_by_b.pop(b)[:, :],
                                                 AF.Copy)
                            nc.scalar.dma_start(
                                out=aggA_d[b * P:(b + 1) * P, :],
                                in_=tmpa[:, :])
                        else:
                            l1_finalize(b)

            l1_round(L1A, 0, h1gA, False)
            l1_round(L1B, L1A["nblocks"] // 8, h1gB, True)

            # ================= LAYER 2 scatter (2 rounds) =================
            ps2 = {}

            def l2_finalize(b):
                psb2 = ps2.pop(b)
                if sched["g2b_nz"]:
                    nc.tensor.matmul(
                        psb2[:, :G2], lhsT=sdeg_s[:1, b * P:(b + 1) * P],
                        rhs=g2b_s[:1, :], start=False, stop=True,
                        skip_group_check=True)
                g2t = qp.tile([P, G2], bf, tag="g2t")
                nc.scalar.activation(g2t[:, :], psb2[:, :G2], AF.Relu,
                                     scale=dinv_s[:, b:b + 1])
                if DBG:
                    g2d = qp.tile([P, G2], f32, tag="g2d", bufs=2)
                    nc.vector.tensor_copy(g2d[:, :], psb2[:, :G2])
                    nc.scalar.dma_start(
                        out=dbg_g2_d[b * P:(b + 1) * P, :], in_=g2d[:, :])
                pstg = psT.tile([P, P], bf, tag="t")
                nc.tensor.transpose(pstg[:G2, :], g2t[:, :], ident[:, :])
                g2T = qp.tile([G2, P], bf, tag="g2T")
                if b % 2 == 0:
                    nc.vector.tensor_copy(g2T[:, :], pstg[:G2, :])
                else:
                    nc.scalar.activation(g2T[:, :], pstg[:G2, :], AF.Copy)
                psf = psB.tile([P, 512], f32, tag="b")
                nc.tensor.matmul(psf[:, :FOUT], lhsT=g2T[:, :],
                                 rhs=fcw_s[:, :], start=True,
                                 stop=not sched["fcb_nz"])
                if sched["fcb_nz"]:
                    nc.tensor.matmul(psf[:, :FOUT], lhsT=ones1[:1, :],
                                     rhs=fcb_s[:1, :], start=False,
                                     stop=True, skip_group_check=True)
                nc.vector.tensor_copy(
                    out_acc[:, b * FOUT:(b + 1) * FOUT], psf[:, :FOUT])

            def l2_round(meta, sb_base, table, is_b):
                for q in range(meta["nblocks"] // 32):
                    r0 = (sb_base + q * 4) * P
                    gt2 = sp.tile([P, 32 * G2], f8t2, tag="gt2", bufs=2)
                    ix2 = sp.tile([P, 32], dt.int32, tag="ix2")
                    nc.sync.dma_start(
                        out=ix2[:, :].rearrange("p (a e) -> p a e", e=8),
                        in_=idx2_d[r0:r0 + 4 * P, :]
                            .rearrange("(a p) e -> p a e", p=P))
                    nc.gpsimd.indirect_dma_start(
                        out=gt2[:, :], out_offset=None, in_=table[:, :],
                        in_offset=bass.IndirectOffsetOnAxis(ap=ix2[:, :],
                                                            axis=0))
                    S2 = sp.tile([P, 32 * P], f8, tag="S2", bufs=2)
                    nc.scalar.dma_start(
                        out=S2[:, :].rearrange("p (a v) -> p a v", v=8 * P),
                        in_=Sm2_d[r0:r0 + 4 * P, :]
                            .rearrange("(a p) v -> p a v", p=P))
                    for j in range(32):
                        g = q * 32 + j
                        b = int(meta["b_of"][g])
                        first = bool(meta["first"][g])
                        last = bool(meta["last"][g])
                        if first:
                            psb2 = psS.tile([P, G1], f32, tag="agg",
                                            name="agg2")
                            ps2[b] = psb2
                            if is_b:
                                nc.tensor.matmul(
                                    psb2[:, :G2], lhsT=ident[:, :],
                                    rhs=agg2_s[:, b * G2:(b + 1) * G2],
                                    start=True, stop=False)
                        psb2 = ps2[b]
                        stop = last and (not sched["g2b_nz"] if is_b else True)
                        nc.tensor.matmul(
                            psb2[:, :G2], lhsT=S2[:, j * P:(j + 1) * P],
                            rhs=gt2[:, j * G2:(j + 1) * G2],
                            start=(first and not is_b), stop=stop)
                        if not last:
                            continue
                        if not is_b:
                            nc.scalar.activation(
                                agg2_s[:, b * G2:(b + 1) * G2],
                                ps2.pop(b)[:, :G2], AF.Copy)
                        else:
                            l2_finalize(b)

            l2_round(L2A, 0, h2gA, False)
            l2_round(L2B, L2A["nblocks"] // 8, h2gB, True)

            # batched log_softmax over all node blocks (logits are tiny:
            # exp without max-shift is safe)
            e_all = qp.tile([P, NB * FOUT], f32, tag="eall", bufs=1)
            nc.scalar.activation(e_all[:, :], out_acc[:, :], AF.Exp)
            sums = qp.tile([P, NB], f32, tag="sums", bufs=1)
            nc.vector.reduce_sum(
                sums[:, :],
                e_all[:, :].rearrange("p (b f) -> p b f", f=FOUT),
                axis=AX.X)
            lns = qp.tile([P, NB], f32, tag="lns", bufs=1)
            nc.scalar.activation(lns[:, :], sums[:, :], AF.Ln)
            res = qp.tile([P, NB * FOUT], f32, tag="eall", bufs=1, name="res")
            nc.vector.tensor_tensor(
                out=res[:, :].rearrange("p (b f) -> p b f", f=FOUT),
                in0=out_acc[:, :].rearrange("p (b f) -> p b f", f=FOUT),
                in1=lns[:, :].unsqueeze(2).to_broadcast([P, NB, FOUT]),
                op=OP.subtract)
            nc.scalar.dma_start(
                out=out_d[:, :].rearrange("(b p) f -> p b f", p=P),
                in_=res[:, :].rearrange("p (b f) -> p b f", f=FOUT))
    return nc


_LAST_EXEC_NS = None
_LAST_RESULT = None


def run(inputs, cfg, trace=False, debug=False):
    global _LAST_EXEC_NS, _LAST_RESULT
    in_maps, sched = host_prep(inputs, cfg)
    nc = build(cfg, sched, debug=debug)
    nc.finalize()
    from concourse import bass_utils
    res = bass_utils.run_bass_kernel_spmd(
        nc, in_maps, core_ids=list(range(cfg.NC)), trace=trace)
    _LAST_EXEC_NS = res.exec_time_ns
    _LAST_RESULT = res
    outs = [np.asarray(res.results[c]["out"])[:cfg.NLOC_RAW]
            for c in range(cfg.NC)]
    return np.concatenate(outs, 0).astype(np.float32)


def kernel(**inputs):
    return run(inputs, _Cfg(**CFG_FULL))


# revision 27
# speedup vs baseline: 1.1099x; 1.1099x over previous
"""Distributed Trainium2 kernel for the GNN message-passing model.

Self-contained: host-side structural prep (sharding, edge sort, index
remap) + Bass/Tile SPMD kernel across 8 NeuronCores.

Math (see reference):
  logits = MLP(x1); m = 0.15 + 0.55*onehot(argmax(logits))
  r1 = (m@W1s.sum(-1))*x2 + m@bp1
  g1 = relu(Dh A Dh (r1@gcn1_w) + gcn1_b); g1 = (m@W12)*g1 + 2e-4*(r1@W13)
  r2 = (m@W2.sum(-1))*g1 + m@bp2
  g2 = relu(Dh A Dh (r2@gcn2_w) + gcn2_b)
  out = log_softmax(g2@fc_w + fc_b)
where Dh = diag(deg^-1/2), deg = in-degree over dst.

Distribution: nodes sharded contiguously over 8 cores. Per GCN layer the
scaled features h' = Dh*h are AllGathered in fp8 (in node chunks, so
comm overlaps the producer pipeline); each core gathers h'[src] for
edges whose dst it owns via indirect DMA and scatter-reduces them with
one-hot matmuls on the TensorEngine (PSUM accumulation per dst block).
The one-hot scatter matrices are generated on-chip (iota==drel on DVE)
instead of streamed from HBM; fp8 DoubleRow matmuls contract 256 edges
per instruction for GCN layer 1.
"""

import numpy as np

P = 128
TAU_HI = 0.7
TAU_LO = 0.15  # (1-0.7)/2


class _Cfg:
    def __init__(self, N, E, F1=768, H=512, G1=256, G2=32, FOUT=40, C=7):
        self.NC = 8
        self.N = N
        self.E = E
        self.NLOC_RAW = N // self.NC
        self.NB = -(-self.NLOC_RAW // P)          # node blocks per core
        self.NLOC = self.NB * P
        assert self.NB % C == 0, (self.NB, C)
        self.C = C                                 # allgather chunks
        self.BPC = self.NB // C                    # blocks per chunk
        self.CH = self.BPC * P                     # chunk nodes
        self.TR = self.NC * self.NLOC              # gathered table rows
        self.CHR = self.NC * self.CH               # rows per chunk in table
        self.F1, self.H, self.G1, self.G2, self.FOUT = F1, H, G1, G2, FOUT
        self.KF1 = F1 // P                         # 6 k-tiles
        self.KH = H // P                           # 4
        self.KG1 = G1 // P                         # 2
        self.SPL1 = max(1, C - 1)                  # L1 round-A src chunks
        self.SPL2 = max(1, C - 3)                  # L2 round-A src chunks
        self.NFREE = min(448, self.CH)             # front free-dim unit
        assert self.CH % self.NFREE == 0
        self.FU = self.CH // self.NFREE            # free units per chunk


CFG_FULL = dict(N=50000, E=800000)


def _to_bf16(x):
    import ml_dtypes
    return np.asarray(x, np.float32).astype(ml_dtypes.bfloat16)


def _to_f8(x):
    import ml_dtypes
    return np.clip(np.asarray(x, np.float32), -240.0, 240.0).astype(
        ml_dtypes.float8_e4m3fn)


def _row_of_node(v, cfg):
    """Gathered-table row for global node id v (vectorized)."""
    c = v // cfg.NLOC_RAW
    s = v - c * cfg.NLOC_RAW
    k = s // cfg.CH
    return k * cfg.CHR + c * cfg.CH + (s - k * cfg.CH)


def host_prep(inputs, cfg):
    """Returns (in_maps, sched). sched is baked into the built graph and
    must be identical for every core (SPMD)."""
    x1 = np.asarray(inputs["x1"], np.float32)
    x2 = np.asarray(inputs["x2"], np.float32)
    ei = np.asarray(inputs["edge_index"])
    src = ei[0].astype(np.int64)
    dst = ei[1].astype(np.int64)
    N, E, NC = cfg.N, cfg.E, cfg.NC
    assert x1.shape[0] == N and src.shape[0] == E

    deg = np.bincount(dst, minlength=N).astype(np.float64)
    dinv = np.where(deg > 0, deg ** -0.5, 0.0).astype(np.float32)
    sdeg = np.sqrt(deg).astype(np.float32)  # 1/dinv where deg>0 else 0

    # ---- per-core edge partition by dst owner, sorted by dst block ----
    owner = dst // cfg.NLOC_RAW
    dloc = dst - owner * cfg.NLOC_RAW
    dblk = dloc // P
    drel_all = (dloc - dblk * P).astype(np.float32)
    rows_all = _row_of_node(src, cfg).astype(np.int32)

    per_core = []
    cnt = np.zeros((NC, cfg.NB), np.int64)
    for c in range(NC):
        sel = np.where(owner == c)[0]
        order = np.argsort(dblk[sel], kind="stable")
        sel = sel[order]
        b_of = dblk[sel]
        bounds = np.searchsorted(b_of, np.arange(cfg.NB + 1))
        lists = []
        for b in range(cfg.NB):
            idxs = sel[bounds[b]:bounds[b + 1]]
            lists.append((rows_all[idxs], drel_all[idxs]))
            cnt[c, b] = len(idxs)
        per_core.append(lists)

    # Two uniform cross-core layouts, each split in 2 rounds by src chunk
    # (round boundary = which AllGather chunks the gathers depend on).
    # pair=True keeps per-block bundle counts even so DoubleRow matmuls
    # can consume bundle pairs belonging to the same dst block.
    def build_layout(split_chunk, pad_mult, pair):
        bounds_k = [0, split_chunk * cfg.CHR, cfg.C * cfg.CHR]
        layout = dict(rounds=[])
        for r in range(2):
            lo, hi = bounds_k[r], bounds_k[r + 1]
            cntr = np.zeros((NC, cfg.NB), np.int64)
            per_rc = []
            for c in range(NC):
                pc = []
                for b in range(cfg.NB):
                    rows, rel = per_core[c][b]
                    m = (rows >= lo) & (rows < hi)
                    pc.append((rows[m] - lo, rel[m]))
                    cntr[c, b] = int(m.sum())
                per_rc.append(pc)
            # layouts must be identical on every core: use the max count
            mx = cntr.max(axis=0)
            if pair:
                Kb = 2 * np.maximum(1, -(-mx // (2 * P))).astype(np.int64)
            else:
                Kb = np.maximum(1, -(-mx // P)).astype(np.int64)
            nb_round = int(Kb.sum())
            pad = (-nb_round) % pad_mult
            nb_round += pad
            b_of = np.concatenate([np.repeat(np.arange(cfg.NB), Kb),
                                   np.full(pad, cfg.NB - 1)])
            first = np.zeros(nb_round, bool)
            last = np.zeros(nb_round, bool)
            off = 0
            for b in range(cfg.NB):
                first[off] = True
                e = off + int(Kb[b])
                if b == cfg.NB - 1:
                    e = nb_round
                last[e - 1] = True
                off += int(Kb[b])
            layout["rounds"].append(dict(Kb=Kb, nblocks=nb_round, b_of=b_of,
                                         first=first, last=last,
                                         per_rc=per_rc))
        return layout

    lay1 = build_layout(cfg.SPL1, 16, True)
    lay2 = build_layout(cfg.SPL2, 32, False)

    # pack per-core gather indices + fp8 one-hot scatter matrices.
    # scale_cols (len NLOC): per-dst-node factor folded into the one-hot
    # columns (used to fold Dh of the dst side into the L2 scatter).
    def pack_layout(layout, c, scale_cols=None):
        idxs, Ss = [], []
        eye = np.eye(P, dtype=np.float32)
        zrow = np.zeros((1, P), np.float32)
        eyez = np.concatenate([eye, zrow], 0)  # row P = padding (all-zero)
        for rr in layout["rounds"]:
            sbs = rr["nblocks"] // 8
            idx = np.zeros((sbs * P, 8), np.int32)
            drl = np.full((sbs * P, 8), P, np.int32)
            bofj = np.zeros((sbs * P, 8), np.int32)
            g = 0
            for b in range(cfg.NB):
                rows, rel = rr["per_rc"][c][b]
                n = len(rows)
                nblk = int(rr["Kb"][b])
                if b == cfg.NB - 1:
                    nblk = rr["nblocks"] - g
                for j in range(nblk):
                    s, jj = g // 8, g % 8
                    bofj[s * P:(s + 1) * P, jj] = int(rr["b_of"][g])
                    e0 = j * P
                    m = min(P, max(0, n - e0))
                    if m > 0:
                        idx[s * P:s * P + m, jj] = rows[e0:e0 + m]
                        drl[s * P:s * P + m, jj] = rel[e0:e0 + m].astype(
                            np.int32)
                    g += 1
            idxs.append(idx)
            S = eyez[drl.reshape(sbs * P, 8)]  # [sbs*P, 8, P]
            if scale_cols is not None:
                S = S * scale_cols[bofj.reshape(sbs * P, 8)[:, :, None] * P +
                                   np.arange(P)[None, None, :]]
            Ss.append(_to_f8(S.reshape(sbs * P, 8 * P)))
        return (np.concatenate(idxs, axis=0), np.concatenate(Ss, axis=0))

    sched = dict(lay1=[dict(nblocks=r["nblocks"], b_of=r["b_of"],
                            first=r["first"], last=r["last"])
                       for r in lay1["rounds"]],
                 lay2=[dict(nblocks=r["nblocks"], b_of=r["b_of"],
                            first=r["first"], last=r["last"])
                       for r in lay2["rounds"]])

    # ---- weights ----
    w1 = np.asarray(inputs["mlp_w1"], np.float32)
    w2 = np.asarray(inputs["mlp_w2"], np.float32)
    w3 = np.asarray(inputs["mlp_w3"], np.float32)
    b1 = np.asarray(inputs["mlp_b1"], np.float32)
    b2 = np.asarray(inputs["mlp_b2"], np.float32)
    b3 = np.asarray(inputs["mlp_b3"], np.float32)
    W1s = np.asarray(inputs["W1"], np.float32).sum(-1)
    W12 = np.asarray(inputs["W12"], np.float32)
    W13 = np.asarray(inputs["W13"], np.float32) * 2e-4
    bp1 = np.asarray(inputs["bp1"], np.float32)
    W2s = np.asarray(inputs["W2"], np.float32).sum(-1)
    bp2 = np.asarray(inputs["bp2"], np.float32)
    g1w = np.asarray(inputs["gcn1_w"], np.float32)
    g1b = np.asarray(inputs["gcn1_b"], np.float32)
    g2w = np.asarray(inputs["gcn2_w"], np.float32)
    g2b = np.asarray(inputs["gcn2_b"], np.float32)
    fcw = np.asarray(inputs["fc_w"], np.float32)
    fcb = np.asarray(inputs["fc_b"], np.float32)

    sched["bp1_nz"] = bool(np.any(bp1 != 0))
    sched["bp2_nz"] = bool(np.any(bp2 != 0))
    sched["g1b_nz"] = bool(np.any(g1b != 0))
    sched["g2b_nz"] = bool(np.any(g2b != 0))
    sched["fcb_nz"] = bool(np.any(fcb != 0))
    sched["b3_nz"] = bool(np.any(b3 != 0))
    sched["w12_one"] = bool(np.allclose(W12, 1.0))

    def pack_lhsT_dr(w, KT, MT):
        # DoubleRow stationary layout: [(kp, m, h)] blocks; k-tile pair
        # (2kp, 2kp+1) of output block m sits adjacent.
        o = np.zeros((P, KT * MT * P), np.float32)
        for kp in range(KT // 2):
            for m in range(MT):
                for h in range(2):
                    o[:, ((kp * MT + m) * 2 + h) * P:
                         ((kp * MT + m) * 2 + h + 1) * P] = \
                        w[(2 * kp + h) * P:(2 * kp + h + 1) * P,
                          m * P:(m + 1) * P]
        return _to_f8(o)

    def pack_rhs(w, KT, F, conv=_to_bf16):
        o = np.zeros((P, KT * F), np.float32)
        for k in range(KT):
            o[:, k * F:(k + 1) * F] = w[k * P:(k + 1) * P, :]
        return conv(o)

    def pack_k3(w, F):
        o = np.zeros((4, F), np.float32)
        o[:3] = w
        return _to_bf16(o)

    w1_p = pack_lhsT_dr(w1, cfg.KF1, cfg.KH)
    w2_p = pack_lhsT_dr(w2, cfg.KH, cfg.KH)
    # logits rhs padded to 16 cols per k-tile so DoubleRow pair stride
    # is 16B-aligned
    w3_p = pack_rhs(np.pad(w3, ((0, 0), (0, 13))), cfg.KH, 16, _to_f8)
    b1_p = b1.reshape(cfg.KH, P).T.copy()
    b2_p = b2.reshape(cfg.KH, P).T.copy()
    b3_p = np.pad(b3, (0, 1)).reshape(1, 4).repeat(P, 0).copy()
    # fused rhs for h1' and z: [768, 256+256]
    gz = np.concatenate([g1w, W13], axis=1)
    gz_p = pack_rhs(gz, cfg.KF1, 2 * cfg.G1, _to_f8)
    g2w_p = pack_rhs(g2w, cfg.KG1, cfg.G2)
    fcw_p = _to_bf16(fcw)
    W1s_p = pack_k3(W1s, cfg.F1)
    bp1_p = pack_k3(bp1, cfg.F1)
    W12_p = pack_k3(W12, cfg.G1)
    W2s_p = pack_k3(W2s, cfg.G1)
    bp2_p = pack_k3(bp2, cfg.G1)
    g1b_p = _to_bf16(g1b.reshape(1, cfg.G1))
    g2b_p = _to_bf16(g2b.reshape(cfg.G2, 1))
    fcb_p = _to_bf16(fcb.reshape(1, cfg.FOUT))

    in_maps = []
    for c in range(NC):
        lo = c * cfg.NLOC_RAW
        hi = lo + cfg.NLOC_RAW
        x1T = np.zeros((cfg.F1, cfg.NLOC), np.float32)
        x1T[:, :cfg.NLOC_RAW] = x1[lo:hi].T
        x2T = np.zeros((cfg.F1, cfg.NLOC), np.float32)
        x2T[:, :cfg.NLOC_RAW] = x2[lo:hi].T
        dinv_t = np.zeros((P, cfg.NB), np.float32)
        dinv_t.T.reshape(-1)[:cfg.NLOC_RAW] = dinv[lo:hi]
        sdeg_r = np.zeros((1, cfg.NLOC), np.float32)
        sdeg_r[0, :cfg.NLOC_RAW] = sdeg[lo:hi]

        ident_np = _to_bf16(np.eye(P, dtype=np.float32))
        dinv_pad = np.zeros(cfg.NLOC, np.float32)
        dinv_pad[:cfg.NLOC_RAW] = dinv[lo:hi]
        idx1, Sm1 = pack_layout(lay1, c)
        idx2, Sm2 = pack_layout(lay2, c, scale_cols=dinv_pad)
        im = {
            "ident": ident_np,
            "x1T": _to_f8(x1T), "x2T": _to_f8(x2T),
            "idx1": idx1, "Sm1": Sm1, "idx2": idx2, "Sm2": Sm2,
            "dinv_t": dinv_t, "sdeg_r": _to_bf16(sdeg_r),
            "w1": w1_p, "w2": w2_p, "w3": w3_p,
            "b1": b1_p, "b2": b2_p, "b3": b3_p,
            "gz": gz_p, "g2w": g2w_p, "fcw": fcw_p,
            "W1s": W1s_p, "bp1": bp1_p, "W12": W12_p, "W2s": W2s_p,
            "bp2": bp2_p, "g1b": g1b_p, "g2b": g2b_p, "fcb": fcb_p,
        }
        in_maps.append(im)
    return in_maps, sched


def build(cfg, sched, debug=False):
    import concourse.bacc as bacc
    import concourse.bass as bass
    import concourse.mybir as mybir
    import concourse.tile as tile

    dt = mybir.dt
    AF = mybir.ActivationFunctionType
    OP = mybir.AluOpType
    AX = mybir.AxisListType
    DR = mybir.MatmulPerfMode.DoubleRow

    nc = bacc.Bacc("TRN2", target_bir_lowering=False, debug=debug)

    NB, C, BPC, CH, NLOC, TR, CHR = (cfg.NB, cfg.C, cfg.BPC, cfg.CH,
                                     cfg.NLOC, cfg.TR, cfg.CHR)
    F1, H, G1, G2, FOUT = cfg.F1, cfg.H, cfg.G1, cfg.G2, cfg.FOUT
    KF1, KH, KG1 = cfg.KF1, cfg.KH, cfg.KG1
    NF, FU = cfg.NFREE, cfg.FU
    L1A, L1B = sched["lay1"]
    L2A, L2B = sched["lay2"]
    SB1 = (L1A["nblocks"] + L1B["nblocks"]) // 8
    SB2T = (L2A["nblocks"] + L2B["nblocks"]) // 8
    SPL1, SPL2 = cfg.SPL1, cfg.SPL2

    bf = dt.bfloat16
    f8 = dt.float8e4
    f32 = dt.float32

    import os
    L1W = 1 if os.environ.get("K_NO_DR_SCAT") else 2
    f8t2 = dt.bfloat16

    dd = {}

    def din(name, shape, dtype):
        dd[name] = nc.declare_dram_parameter(name, list(shape), dtype,
                                             isOutput=False)
        return dd[name]

    x1T_d = din("x1T", [F1, NLOC], f8)
    x2T_d = din("x2T", [F1, NLOC], f8)
    idx1_d = din("idx1", [SB1 * P, 8], dt.int32)
    Sm1_d = din("Sm1", [SB1 * P, 8 * P], f8)
    idx2_d = din("idx2", [SB2T * P, 8], dt.int32)
    Sm2_d = din("Sm2", [SB2T * P, 8 * P], f8)
    dinv_d = din("dinv_t", [P, NB], f32)
    sdeg_d = din("sdeg_r", [1, NLOC], bf)
    w1_d = din("w1", [P, KF1 * KH * P], f8)
    w2_d = din("w2", [P, KH * KH * P], f8)
    w3_d = din("w3", [P, KH * 16], f8)
    b1_d = din("b1", [P, KH], f32)
    b2_d = din("b2", [P, KH], f32)
    b3_d = din("b3", [P, 4], f32)
    gz_d = din("gz", [P, KF1 * 2 * G1], f8)
    g2w_d = din("g2w", [P, KG1 * G2], bf)
    fcw_d = din("fcw", [G2, FOUT], bf)
    W1s_d = din("W1s", [4, F1], bf)
    bp1_d = din("bp1", [4, F1], bf)
    W12_d = din("W12", [4, G1], bf)
    W2s_d = din("W2s", [4, G1], bf)
    bp2_d = din("bp2", [4, G1], bf)
    g1b_d = din("g1b", [1, G1], bf)
    g2b_d = din("g2b", [G2, 1], bf)
    fcb_d = din("fcb", [1, FOUT], bf)
    ident_d = din("ident", [P, P], bf)
    out_d = nc.declare_dram_parameter("out", [NLOC, FOUT], f32, isOutput=True)
    DBG = bool(os.environ.get("K_DEBUG_DUMP"))
    if DBG:
        dbg_r2_d = nc.declare_dram_parameter("dbg_r2", [NLOC, G1], f32,
                                             isOutput=True)
        dbg_g2_d = nc.declare_dram_parameter("dbg_g2", [NLOC, G2], f32,
                                             isOutput=True)

    with tile.TileContext(nc) as tc:
        with (
            tc.tile_pool(name="const", bufs=1) as cp,
            tc.tile_pool(name="front", bufs=2) as fp,
            tc.tile_pool(name="scat", bufs=3) as sp,
            tc.tile_pool(name="fin", bufs=2) as qp,
            tc.tile_pool(name="psG", bufs=2, space="PSUM") as psG,
            tc.tile_pool(name="psS", bufs=2, space="PSUM") as psS,
            tc.tile_pool(name="psB", bufs=2, space="PSUM") as psB,
            tc.tile_pool(name="psT", bufs=2, space="PSUM") as psT,
            tc.tile_pool(name="dram", bufs=1, space="DRAM") as dp,
        ):
            def load(dr, shape, dtype, name):
                t = cp.tile(shape, dtype, tag=name)
                nc.sync.dma_start(out=t[:, :], in_=dr[:, :])
                return t

            w1_s = load(w1_d, [P, KF1 * KH * P], f8, "w1")
            w2_s = load(w2_d, [P, KH * KH * P], f8, "w2")
            w3_s = load(w3_d, [P, KH * 16], f8, "w3")
            b1_s = load(b1_d, [P, KH], f32, "b1")
            b2_s = load(b2_d, [P, KH], f32, "b2")
            b3_s = load(b3_d, [P, 4], f32, "b3")
            gz_s = load(gz_d, [P, KF1 * 2 * G1], f8, "gz")
            g2w_s = load(g2w_d, [P, KG1 * G2], bf, "g2w")
            fcw_s = load(fcw_d, [G2, FOUT], bf, "fcw")
            W1s_s = load(W1s_d, [4, F1], bf, "W1s")
            bp1_s = load(bp1_d, [4, F1], bf, "bp1")
            W12_s = load(W12_d, [4, G1], bf, "W12")
            W2s_s = load(W2s_d, [4, G1], bf, "W2s")
            bp2_s = load(bp2_d, [4, G1], bf, "bp2")
            g1b_s = load(g1b_d, [1, G1], bf, "g1b")
            g2b_s = load(g2b_d, [G2, 1], bf, "g2b")
            fcb_s = load(fcb_d, [1, FOUT], bf, "fcb")
            dinv_s = load(dinv_d, [P, NB], f32, "dinv")
            sdeg_s = load(sdeg_d, [1, NLOC], bf, "sdeg")

            ident = load(ident_d, [P, P], bf, "ident")
            ones1 = cp.tile([1, P], bf, tag="ones1")
            nc.vector.memset(ones1[:, :], 1.0)

            mT_s = cp.tile([4, NLOC], bf, tag="mT")
            out_acc = cp.tile([P, NB * FOUT], f32, tag="oacc")
            z_s = cp.tile([P, NB * G1], bf, tag="z")

            h1b_all = dp.tile([NLOC, G1], f8, tag="h1b")
            ag1_after = {2: (0, 3), 5: (3, 6), C - 1: (6, C)}
            h2bA = dp.tile([SPL2 * CH, G2], f8t2, tag="h2bA")
            h2bB = dp.tile([(C - SPL2) * CH, G2], f8t2, tag="h2bB")
            h1gA = dp.tile([SPL1 * CHR, G1], f8, tag="h1gA")
            h1gB = dp.tile([(C - SPL1) * CHR, G1], f8, tag="h1gB")
            h2gA = dp.tile([SPL2 * CHR, G2], f8t2, tag="h2gA")
            h2gB = dp.tile([(C - SPL2) * CHR, G2], f8t2, tag="h2gB")
            aggA_d = dp.tile([NLOC, G1], bf, tag="aggA")
            agg2_s = cp.tile([G2, NB * P], bf, tag="agg2")

            def pair_k(t, width, kp, c0, cn):
                """[P, 2, cn] view of k-tile pair (2kp, 2kp+1), cols
                c0:c0+cn, from a [P, K*width] k-major tile."""
                return t[:, :].rearrange("p (k n) -> p k n", n=width)[
                    :, 2 * kp:2 * kp + 2, c0:c0 + cn]

            # ================= FRONT (per chunk) =================
            for k in range(C):
                n0 = k * CH
                x1c = fp.tile([P, KF1 * CH], f8, tag="x1c")
                nc.sync.dma_start(
                    out=x1c[:, :].rearrange("p (a n) -> p a n", n=CH),
                    in_=x1T_d[:, n0:n0 + CH].rearrange("(a p) n -> p a n", p=P))
                x2c = fp.tile([P, KF1 * CH], f8, tag="x2c", bufs=1)
                nc.sync.dma_start(
                    out=x2c[:, :].rearrange("p (a n) -> p a n", n=CH),
                    in_=x2T_d[:, n0:n0 + CH].rearrange("(a p) n -> p a n", p=P))

                h1T = fp.tile([P, KH * CH], f8, tag="h1T", bufs=1)
                for u in range(FU):
                    for m in range(KH):
                        ps = psG.tile([P, 512], f32, tag="g")
                        for kp in range(KF1 // 2):
                            nc.tensor.matmul(
                                ps[:, :NF],
                                lhsT=w1_s[:, :].rearrange(
                                    "p (a x) -> p a x", x=P)[
                                    :, (kp * KH + m) * 2:(kp * KH + m) * 2 + 2, :],
                                rhs=pair_k(x1c, CH, kp, u * NF, NF),
                                start=(kp == 0), stop=(kp == KF1 // 2 - 1),
                                perf_mode=DR)
                        nc.scalar.activation(
                            h1T[:, m * CH + u * NF:m * CH + u * NF + NF],
                            ps[:, :NF], AF.Relu, bias=b1_s[:, m:m + 1])
                h2T = fp.tile([P, KH * CH], f8, tag="h2T", bufs=1)
                for u in range(FU):
                    for m in range(KH):
                        ps = psG.tile([P, 512], f32, tag="g")
                        for kp in range(KH // 2):
                            nc.tensor.matmul(
                                ps[:, :NF],
                                lhsT=w2_s[:, :].rearrange(
                                    "p (a x) -> p a x", x=P)[
                                    :, (kp * KH + m) * 2:(kp * KH + m) * 2 + 2, :],
                                rhs=pair_k(h1T, CH, kp, u * NF, NF),
                                start=(kp == 0), stop=(kp == KH // 2 - 1),
                                perf_mode=DR)
                        nc.scalar.activation(
                            h2T[:, m * CH + u * NF:m * CH + u * NF + NF],
                            ps[:, :NF], AF.Relu, bias=b2_s[:, m:m + 1])

                mmc = fp.tile([P, BPC * 3], bf, tag="mmc")
                for nb in range(BPC):
                    psl = psB.tile([P, 512], f32, tag="b")
                    for kp in range(KH // 2):
                        nc.tensor.matmul(
                            psl[:, :16],
                            lhsT=pair_k(h2T, CH, kp, nb * P, P),
                            rhs=pair_k(w3_s, 16, kp, 0, 16),
                            start=(kp == 0), stop=(kp == KH // 2 - 1),
                            perf_mode=DR)
                    lg = fp.tile([P, 3], f32, tag="lg")
                    if sched["b3_nz"]:
                        nc.vector.tensor_add(lg[:, :], psl[:, :3], b3_s[:, :3])
                    else:
                        nc.vector.tensor_copy(lg[:, :], psl[:, :3])
                    rmax = fp.tile([P, 1], f32, tag="rmax")
                    nc.vector.reduce_max(rmax[:, :], lg[:, :], axis=AX.X)
                    mm = fp.tile([P, 3], bf, tag="mm")
                    nc.vector.tensor_scalar(
                        mm[:, :], lg[:, :], rmax[:, :1], None, OP.is_equal)
                    nc.scalar.activation(mmc[:, nb * 3:(nb + 1) * 3],
                                         mm[:, :], AF.Copy,
                                         bias=TAU_LO, scale=TAU_HI - TAU_LO)
                for nb in range(BPC):
                    b_glob = k * BPC + nb
                    pst = psT.tile([P, P], bf, tag="t")
                    nc.tensor.transpose(pst[:3, :],
                                        mmc[:, nb * 3:(nb + 1) * 3],
                                        ident[:, :])
                    nc.vector.tensor_copy(
                        mT_s[:3, b_glob * P:(b_glob + 1) * P], pst[:3, :])

                r1T = fp.tile([P, KF1 * CH], f8, tag="r1T")
                for u in range(FU):
                    for f in range(KF1):
                        psr = psG.tile([P, 512], f32, tag="g")
                        nc.tensor.matmul(
                            psr[:, :NF], lhsT=W1s_s[:3, f * P:(f + 1) * P],
                            rhs=mT_s[:3, n0 + u * NF:n0 + u * NF + NF],
                            start=True, stop=True)
                        if sched["bp1_nz"]:
                            psr2 = psB.tile([P, 512], f32, tag="b")
                            nc.tensor.matmul(
                                psr2[:, :NF], lhsT=bp1_s[:3, f * P:(f + 1) * P],
                                rhs=mT_s[:3, n0 + u * NF:n0 + u * NF + NF],
                                start=True, stop=True)
                            tmp = fp.tile([P, NF], f32, tag="r1tmp")
                            nc.vector.tensor_mul(
                                tmp[:, :], psr[:, :NF],
                                x2c[:, f * CH + u * NF:f * CH + u * NF + NF])
                            nc.vector.tensor_add(
                                r1T[:, f * CH + u * NF:f * CH + u * NF + NF],
                                tmp[:, :], psr2[:, :NF])
                        else:
                            nc.vector.tensor_mul(
                                r1T[:, f * CH + u * NF:f * CH + u * NF + NF],
                                psr[:, :NF],
                                x2c[:, f * CH + u * NF:f * CH + u * NF + NF])

                # fused h1' | z: [r1 @ (g1w | 2e-4*W13)] per node block
                for nb in range(BPC):
                    b_glob = k * BPC + nb
                    psh = psG.tile([P, 512], f32, tag="g")
                    for kp in range(KF1 // 2):
                        nc.tensor.matmul(
                            psh[:, :],
                            lhsT=pair_k(r1T, CH, kp, nb * P, P),
                            rhs=pair_k(gz_s, 2 * G1, kp, 0, 2 * G1),
                            start=(kp == 0), stop=(kp == KF1 // 2 - 1),
                            perf_mode=DR)
                    h1p = fp.tile([P, G1], f8, tag="h1p")
                    nc.scalar.activation(h1p[:, :], psh[:, :G1], AF.Copy,
                                         scale=dinv_s[:, b_glob:b_glob + 1])
                    nc.scalar.dma_start(
                        out=h1b_all[(n0 + nb * P):(n0 + (nb + 1) * P), :],
                        in_=h1p[:, :])
                    nc.scalar.activation(
                        z_s[:, b_glob * G1:(b_glob + 1) * G1],
                        psh[:, G1:2 * G1], AF.Copy)

                if k in ag1_after:
                    k0, k1 = ag1_after[k]
                    agt = (h1gA[k0 * CHR:k1 * CHR, :] if k1 <= SPL1 else
                           h1gB[0:CHR, :])
                    nc.gpsimd.collective_compute(
                        "AllGather", OP.bypass,
                        replica_groups=[list(range(cfg.NC))],
                        ins=[h1b_all[k0 * CH:k1 * CH, :].opt()],
                        outs=[agt.opt()])

            # ================= LAYER 1 scatter (2 rounds) =================
            ps_by_b = {}

            def l1_finalize(b):
                psb = ps_by_b.pop(b)
                if sched["g1b_nz"]:
                    nc.tensor.matmul(
                        psb[:, :], lhsT=sdeg_s[:1, b * P:(b + 1) * P],
                        rhs=g1b_s[:1, :], start=False, stop=True,
                        skip_group_check=True)
                g1r = qp.tile([P, G1], bf, tag="g1r", bufs=3)
                nc.scalar.activation(g1r[:, :], psb[:, :], AF.Relu,
                                     scale=dinv_s[:, b:b + 1])
                psmw = psB.tile([P, 512], f32, tag="b")
                if not sched["w12_one"]:
                    nc.tensor.matmul(psmw[:, :G1],
                                     lhsT=mT_s[:3, b * P:(b + 1) * P],
                                     rhs=W12_s[:3, :], start=True, stop=True)
                nc.tensor.matmul(psmw[:, G1:2 * G1],
                                 lhsT=mT_s[:3, b * P:(b + 1) * P],
                                 rhs=W2s_s[:3, :], start=True, stop=True)
                if sched["w12_one"]:
                    # W12 all-ones and sum(m) == 1 -> (m@W12)*g1 == g1
                    g1v = qp.tile([P, G1], bf, tag="g1v", bufs=3)
                    nc.vector.tensor_add(g1v[:, :], g1r[:, :],
                                         z_s[:, b * G1:(b + 1) * G1])
                else:
                    g1t = qp.tile([P, G1], bf, tag="g1t", bufs=3)
                    nc.vector.tensor_mul(g1t[:, :], g1r[:, :], psmw[:, :G1])
                    g1v = qp.tile([P, G1], bf, tag="g1v", bufs=3)
                    nc.vector.tensor_add(g1v[:, :], g1t[:, :],
                                         z_s[:, b * G1:(b + 1) * G1])
                r2 = qp.tile([P, G1], bf, tag="r2", bufs=3)
                if sched["bp2_nz"]:
                    psm3 = psB.tile([P, 512], f32, tag="b")
                    nc.tensor.matmul(psm3[:, :G1],
                                     lhsT=mT_s[:3, b * P:(b + 1) * P],
                                     rhs=bp2_s[:3, :], start=True, stop=True)
                    r2u = qp.tile([P, G1], bf, tag="r2u")
                    nc.vector.tensor_mul(r2u[:, :], g1v[:, :],
                                         psmw[:, G1:2 * G1])
                    r2v = qp.tile([P, G1], bf, tag="r2v")
                    nc.vector.tensor_add(r2v[:, :], r2u[:, :], psm3[:, :G1])
                    nc.vector.tensor_scalar(r2[:, :], r2v[:, :],
                                            dinv_s[:, b:b + 1], None, OP.mult)
                else:
                    nc.vector.scalar_tensor_tensor(
                        out=r2[:, :], in0=g1v[:, :],
                        scalar=dinv_s[:, b:b + 1],
                        in1=psmw[:, G1:2 * G1], op0=OP.mult, op1=OP.mult)
                if DBG:
                    r2d = qp.tile([P, G1], f32, tag="r2d", bufs=2)
                    nc.vector.tensor_copy(r2d[:, :], r2[:, :])
                    nc.scalar.dma_start(
                        out=dbg_r2_d[b * P:(b + 1) * P, :], in_=r2d[:, :])
                r2T = qp.tile([P, KG1 * P], bf, tag="r2T", bufs=3)
                for f in range(KG1):
                    pst = psT.tile([P, P], bf, tag="t")
                    nc.tensor.transpose(pst[:, :], r2[:, f * P:(f + 1) * P],
                                        ident[:, :])
                    nc.vector.tensor_copy(r2T[:, f * P:(f + 1) * P],
                                          pst[:, :])
                psh2 = psB.tile([P, 512], f32, tag="b")
                for f in range(KG1):
                    nc.tensor.matmul(
                        psh2[:, :G2], lhsT=r2T[:, f * P:(f + 1) * P],
                        rhs=g2w_s[:, f * G2:(f + 1) * G2],
                        start=(f == 0), stop=(f == KG1 - 1))
                h2p = qp.tile([P, G2], f8t2, tag="h2p", bufs=3)
                nc.vector.tensor_copy(h2p[:, :], psh2[:, :G2])
                if b < SPL2 * BPC:
                    nc.scalar.dma_start(
                        out=h2bA[b * P:(b + 1) * P, :], in_=h2p[:, :])
                    if b == SPL2 * BPC - 1:
                        nc.gpsimd.collective_compute(
                            "AllGather", OP.bypass,
                            replica_groups=[list(range(cfg.NC))],
                            ins=[h2bA[:, :].opt()], outs=[h2gA[:, :].opt()])
                else:
                    bb = b - SPL2 * BPC
                    nc.scalar.dma_start(
                        out=h2bB[bb * P:(bb + 1) * P, :], in_=h2p[:, :])
                    if b == NB - 1:
                        nc.gpsimd.collective_compute(
                            "AllGather", OP.bypass,
                            replica_groups=[list(range(cfg.NC))],
                            ins=[h2bB[:, :].opt()], outs=[h2gB[:, :].opt()])

            def l1_round(meta, sb_base, table, is_b):
                for s_loc in range(meta["nblocks"] // 16):
                    r0 = (sb_base + s_loc * 2) * P
                    gt = sp.tile([P, 16 * G1], f8, tag="gt1")
                    ix = sp.tile([P, 16], dt.int32, tag="ix1")
                    nc.sync.dma_start(
                        out=ix[:, :].rearrange("p (a e) -> p a e", e=8),
                        in_=idx1_d[r0:r0 + 2 * P, :]
                            .rearrange("(a p) e -> p a e", p=P))
                    nc.gpsimd.indirect_dma_start(
                        out=gt[:, :], out_offset=None, in_=table[:, :],
                        in_offset=bass.IndirectOffsetOnAxis(ap=ix[:, :],
                                                            axis=0))
                    Ssb = sp.tile([P, 16 * P], f8, tag="S1", bufs=2)
                    nc.scalar.dma_start(
                        out=Ssb[:, :].rearrange("p (a v) -> p a v", v=8 * P),
                        in_=Sm1_d[r0:r0 + 2 * P, :]
                            .rearrange("(a p) v -> p a v", p=P))
                    for j2 in range(8):
                        g = s_loc * 16 + j2 * 2
                        b = int(meta["b_of"][g])
                        first = bool(meta["first"][g])
                        last = bool(meta["last"][g + 1])
                        if first:
                            psb = psS.tile([P, G1], f32, tag="agg",
                                           name="agg1")
                            ps_by_b[b] = psb
                            if is_b:
                                rel = sp.tile([P, G1], bf, tag="rel")
                                nc.sync.dma_start(
                                    out=rel[:, :],
                                    in_=aggA_d[b * P:(b + 1) * P, :])
                                nc.tensor.matmul(psb[:, :], lhsT=ident[:, :],
                                                 rhs=rel[:, :], start=True,
                                                 stop=False)
                        psb = ps_by_b[b]
                        stop = last and (not sched["g1b_nz"] if is_b else True)
                        nc.tensor.matmul(
                            psb[:, :],
                            lhsT=Ssb[:, :].rearrange(
                                "p (a v) -> p a v", v=P)[
                                :, 2 * j2:2 * j2 + 2, :],
                            rhs=gt[:, :].rearrange(
                                "p (a v) -> p a v", v=G1)[
                                :, 2 * j2:2 * j2 + 2, :],
                            start=(first and not is_b), stop=stop,
                            perf_mode=DR)
                        if not last:
                            continue
                        if not is_b:
                            tmpa = qp.tile([P, G1], bf, tag="tmpa")
                            nc.scalar.activation(tmpa[:, :],
                                                 ps_by_b.pop(b)[:, :],
                                                 AF.Copy)
                            nc.scalar.dma_start(
                                out=aggA_d[b * P:(b + 1) * P, :],
                                in_=tmpa[:, :])
                        else:
                            l1_finalize(b)

            l1_round(L1A, 0, h1gA, False)
            l1_round(L1B, L1A["nblocks"] // 8, h1gB, True)

            # ================= LAYER 2 scatter (2 rounds) =================
            ps2 = {}

            def l2_finalize(b):
                psb2 = ps2.pop(b)
                g2T = qp.tile([G2, P], bf, tag="g2T")
                if sched["g2b_nz"]:
                    nc.scalar.activation(g2T[:, :], psb2[:, :], AF.Relu,
                                         bias=g2b_s[:, :1])
                else:
                    nc.scalar.activation(g2T[:, :], psb2[:, :], AF.Relu)
                if DBG:
                    g2d = qp.tile([G2, P], f32, tag="g2d", bufs=2)
                    nc.vector.tensor_copy(g2d[:, :], psb2[:, :])
                    nc.scalar.dma_start(
                        out=dbg_g2_d[b * P:(b + 1) * P, :],
                        in_=g2d[:, :].rearrange("g p -> p g"))
                psf = psB.tile([P, 512], f32, tag="b")
                nc.tensor.matmul(psf[:, :FOUT], lhsT=g2T[:, :],
                                 rhs=fcw_s[:, :], start=True,
                                 stop=not sched["fcb_nz"])
                if sched["fcb_nz"]:
                    nc.tensor.matmul(psf[:, :FOUT], lhsT=ones1[:1, :],
                                     rhs=fcb_s[:1, :], start=False,
                                     stop=True, skip_group_check=True)
                nc.vector.tensor_copy(
                    out_acc[:, b * FOUT:(b + 1) * FOUT], psf[:, :FOUT])

            def l2_round(meta, sb_base, table, is_b):
                for q in range(meta["nblocks"] // 32):
                    r0 = (sb_base + q * 4) * P
                    gt2 = sp.tile([P, 32 * G2], f8t2, tag="gt2", bufs=2)
                    ix2 = sp.tile([P, 32], dt.int32, tag="ix2")
                    nc.sync.dma_start(
                        out=ix2[:, :].rearrange("p (a e) -> p a e", e=8),
                        in_=idx2_d[r0:r0 + 4 * P, :]
                            .rearrange("(a p) e -> p a e", p=P))
                    nc.gpsimd.indirect_dma_start(
                        out=gt2[:, :], out_offset=None, in_=table[:, :],
                        in_offset=bass.IndirectOffsetOnAxis(ap=ix2[:, :],
                                                            axis=0))
                    S2 = sp.tile([P, 32 * P], f8, tag="S2", bufs=2)
                    nc.scalar.dma_start(
                        out=S2[:, :].rearrange("p (a v) -> p a v", v=8 * P),
                        in_=Sm2_d[r0:r0 + 4 * P, :]
                            .rearrange("(a p) v -> p a v", p=P))
                    for j in range(32):
                        g = q * 32 + j
                        b = int(meta["b_of"][g])
                        first = bool(meta["first"][g])
                        last = bool(meta["last"][g])
                        if first:
                            psb2 = psS.tile([G2, P], f32, tag="agg",
                                            name="agg2")
                            ps2[b] = psb2
                            if is_b:
                                nc.tensor.matmul(
                                    psb2[:, :], lhsT=ident[:G2, :G2],
                                    rhs=agg2_s[:, b * P:(b + 1) * P],
                                    start=True, stop=False)
                        psb2 = ps2[b]
                        nc.tensor.matmul(
                            psb2[:, :], lhsT=gt2[:, j * G2:(j + 1) * G2],
                            rhs=S2[:, j * P:(j + 1) * P],
                            start=(first and not is_b), stop=last)
                        if not last:
                            continue
                        if not is_b:
                            nc.vector.tensor_copy(
                                agg2_s[:, b * P:(b + 1) * P],
                                ps2.pop(b)[:, :])
                        else:
                            l2_finalize(b)

            l2_round(L2A, 0, h2gA, False)
            l2_round(L2B, L2A["nblocks"] // 8, h2gB, True)

            # batched log_softmax over all node blocks (logits are tiny:
            # exp without max-shift is safe)
            e_all = qp.tile([P, NB * FOUT], f32, tag="eall", bufs=1)
            nc.scalar.activation(e_all[:, :], out_acc[:, :], AF.Exp)
            sums = qp.tile([P, NB], f32, tag="sums", bufs=1)
            nc.vector.reduce_sum(
                sums[:, :],
                e_all[:, :].rearrange("p (b f) -> p b f", f=FOUT),
                axis=AX.X)
            lns = qp.tile([P, NB], f32, tag="lns", bufs=1)
            nc.scalar.activation(lns[:, :], sums[:, :], AF.Ln)
            res = qp.tile([P, NB * FOUT], f32, tag="eall", bufs=1, name="res")
            nc.vector.tensor_tensor(
                out=res[:, :].rearrange("p (b f) -> p b f", f=FOUT),
                in0=out_acc[:, :].rearrange("p (b f) -> p b f", f=FOUT),
                in1=lns[:, :].unsqueeze(2).to_broadcast([P, NB, FOUT]),
                op=OP.subtract)
            nc.scalar.dma_start(
                out=out_d[:, :].rearrange("(b p) f -> p b f", p=P),
                in_=res[:, :].rearrange("p (b f) -> p b f", f=FOUT))
    return nc


_LAST_EXEC_NS = None
_LAST_RESULT = None


def run(inputs, cfg, trace=False, debug=False):
    global _LAST_EXEC_NS, _LAST_RESULT
    in_maps, sched = host_prep(inputs, cfg)
    nc = build(cfg, sched, debug=debug)
    nc.finalize()
    from concourse import bass_utils
    res = bass_utils.run_bass_kernel_spmd(
        nc, in_maps, core_ids=list(range(cfg.NC)), trace=trace)
    _LAST_EXEC_NS = res.exec_time_ns
    _LAST_RESULT = res
    outs = [np.asarray(res.results[c]["out"])[:cfg.NLOC_RAW]
            for c in range(cfg.NC)]
    return np.concatenate(outs, 0).astype(np.float32)


def kernel(**inputs):
    return run(inputs, _Cfg(**CFG_FULL))
